# revision 25
# baseline (speedup 1.0000x reference)
"""Multi-head self-attention Bass kernel for 8 TRN2 NeuronCores.

Problem: B=8, N=1024, C=1024, H=16, D=64, fp32.
  qkv = x @ w_qkv.T ; split to q,k,v per head
  attn = softmax(q k^T / sqrt(D)) ; out = attn @ v ; y = out @ w_proj.T + b_proj

Sharding: data-parallel over batch -- core b computes batch element b end to
end.  No collectives.

Per-core dataflow:
  phase 1: qT/kT slabs [o_part, n] (bf16) and v slabs [n_part, per-head 65
           cols] (bf16; 64 v columns + a ones column so the AV matmul also
           produces the softmax denominator in PSUM row 64).
           QKV/proj matmuls run in float32r (full-rate, ~tf32 precision).
  phase 2: per head-pair (rows 0:64 / 64:128 of a slab): scoresT[m,n] =
           kT^T qT (two K=64 matmuls in disjoint PE row groups run
           concurrently), exp via ACT -> bf16.  ACT is the bottleneck here.
  phase 3: out'[d,n] = v'^T @ exp (K=128 over m, accumulated); row 64 =
           softmax denominator; divide rows 0..63 by it (reciprocal +
           DRAM-bounce partition broadcast + DVE multiply).
  phase 4: y = projT^T @ w_projT + b_proj (float32r).
"""

import os
import sys

sys.path.insert(0, "/opt/trn_rl_repo")

import numpy as np

B, N, C = 8, 1024, 1024
H = 16
D = C // H  # 64
SCALE = D ** -0.5  # 0.125
P = 128
CT = C // P  # 8 contraction tiles of 128
NCH = N // 512  # 2 free-dim chunks of 512

_CACHE = {}

LAST_EXEC_NS = None


def _build():
    import concourse.bacc as bacc
    import concourse.tile as tile
    from concourse import mybir

    fp32 = mybir.dt.float32
    fp32r = mybir.dt.float32r
    bf16 = mybir.dt.bfloat16
    AFT = mybir.ActivationFunctionType

    nc = bacc.Bacc(
        "TRN2",
        target_bir_lowering=False,
        debug=False,
        enable_asserts=False,
        num_devices=8,
    )
    xT = nc.dram_tensor("xT", [C, N], fp32r, kind="ExternalInput")
    wqkvT = nc.dram_tensor("wqkvT", [C, 3 * C], fp32r, kind="ExternalInput")
    wprojT = nc.dram_tensor("wprojT", [C, C], fp32r, kind="ExternalInput")
    bproj = nc.dram_tensor("bproj", [C], fp32, kind="ExternalInput")
    y = nc.dram_tensor("y", [N, C], fp32, kind="ExternalOutput")

    with tile.TileContext(nc) as tc:
        with (
            tc.tile_pool(name="consts", bufs=1) as consts,
            tc.tile_pool(name="xp", bufs=8) as xp,
            tc.tile_pool(name="wq", bufs=16) as wq,
            tc.tile_pool(name="qt", bufs=8) as qtp,
            tc.tile_pool(name="kt", bufs=8) as ktp,
            tc.tile_pool(name="vp", bufs=8) as vp,
            tc.tile_pool(name="ex", bufs=17) as exp_pool,
            tc.tile_pool(name="pj", bufs=1) as pjp,
            tc.tile_pool(name="sm", bufs=2) as small,
            tc.tile_pool(name="bc", bufs=3) as bcp,
            tc.tile_pool(name="tm", bufs=2) as tmpp,
            tc.tile_pool(name="ot", bufs=2) as otp,
            tc.tile_pool(name="dscr", bufs=8, space="DRAM") as dscr,
            tc.tile_pool(name="ps", bufs=2, space="PSUM") as psum,
            tc.tile_pool(name="pav", bufs=4, space="PSUM") as psav,
        ):
            # x^T resident: slab ci holds rows c in [128ci, 128ci+128)
            xts = [xp.tile([P, N], fp32r, name=f"xt{i}", tag="xt") for i in range(CT)]

            def load_x(nch):
                for ci in range(CT):
                    nc.sync.dma_start(
                        xts[ci][:, nch * 512 : (nch + 1) * 512],
                        xT.ap()[
                            ci * P : (ci + 1) * P, nch * 512 : (nch + 1) * 512
                        ],
                    )

            bb = consts.tile([P, C], fp32)

            # ---- phase 1 helpers
            qts = [None] * 8
            kts = [None] * 8

            def load_w(oblk):
                wts = []
                for ci in range(CT):
                    wt = wq.tile([P, 512], fp32r, name="wt", tag="wt")
                    nc.sync.dma_start(
                        wt[:],
                        wqkvT.ap()[
                            ci * P : (ci + 1) * P, oblk * 512 : (oblk + 1) * 512
                        ],
                    )
                    wts.append(wt)
                return wts

            def emit_qk_slab(s, wts, ss):
                slab = (qtp if s < 8 else ktp).tile(
                    [P, N], bf16, name="slab", tag="slab"
                )
                if s < 8:
                    qts[s] = slab
                else:
                    kts[s - 8] = slab
                ps = psum.tile([P, N], fp32)
                for nch in range(NCH):
                    for ci in range(CT):
                        nc.tensor.matmul(
                            ps[:, nch * 512 : (nch + 1) * 512],
                            lhsT=wts[ci][:, ss * P : (ss + 1) * P],
                            rhs=xts[ci][:, nch * 512 : (nch + 1) * 512],
                            start=(ci == 0),
                            stop=(ci == CT - 1),
                        )
                nc.vector.tensor_copy(slab[:], ps[:])

            vslabs = []
            vviews = []

            def emit_v():
                for mi in range(CT):
                    vs = vp.tile([P, H * 65], bf16, name="vs", tag="vs")
                    vv = vs[:].rearrange("p (h e) -> p h e", e=65)
                    nc.gpsimd.memset(vv[:, :, 64:65], 1.0)
                    vslabs.append(vs)
                    vviews.append(vv)
                vwts = []
                for vblk in range(2):  # v o-chunks of 512
                    wts = []
                    for ci in range(CT):
                        wt = wq.tile([P, 512], fp32r, name="wt", tag="wt")
                        nc.sync.dma_start(
                            wt[:],
                            wqkvT.ap()[
                                ci * P : (ci + 1) * P,
                                2048 + vblk * 512 : 2048 + (vblk + 1) * 512,
                            ],
                        )
                        wts.append(wt)
                    vwts.append(wts)
                for mi in range(CT):
                    ps = psum.tile([P, N], fp32)
                    for vblk in range(2):
                        for ci in range(CT):
                            nc.tensor.matmul(
                                ps[:, vblk * 512 : (vblk + 1) * 512],
                                lhsT=xts[ci][:, mi * P : (mi + 1) * P],
                                rhs=vwts[vblk][ci][:],
                                start=(ci == 0),
                                stop=(ci == CT - 1),
                            )
                    nc.vector.tensor_copy(
                        vviews[mi][:, :, 0:64],
                        ps[:].rearrange("p (hh d) -> p hh d", d=64),
                    )

            # proj-input slabs [c-chunk 128, n] fp32r (normalized out^T)
            pjs = [pjp.tile([P, N], fp32r, name=f"pj{i}") for i in range(CT)]

            # ---- phases 2+3 per head pair, software-pipelined by one pair
            def emit_scores_exp(s):
                # heads 2s (rows 0:64) and 2s+1 (rows 64:128) of slab s
                ets = {0: [], 64: []}
                for mi in range(CT):
                    for rowlo in (0, 64):
                        et = exp_pool.tile([P, N], bf16, name="et", tag="et")
                        ets[rowlo].append(et)
                    for nch in range(NCH):
                        for rowlo in (0, 64):
                            ps = psum.tile([P, 512], fp32)
                            nc.tensor.matmul(
                                ps[:],
                                lhsT=kts[s][
                                    rowlo : rowlo + 64, mi * P : (mi + 1) * P
                                ],
                                rhs=qts[s][
                                    rowlo : rowlo + 64,
                                    nch * 512 : (nch + 1) * 512,
                                ],
                                start=True,
                                stop=True,
                            )
                            nc.scalar.activation(
                                ets[rowlo][mi][:, nch * 512 : (nch + 1) * 512],
                                ps[:],
                                AFT.Exp,
                                scale=SCALE,
                            )
                return ets

            def emit_av_div(s, ets):
                avs = {}
                for rowlo in (0, 64):
                    for nch in range(NCH):
                        avs[(rowlo, nch)] = psav.tile(
                            [65, 512], fp32, name="av", tag="av"
                        )
                for mi in range(CT):
                    for rowlo in (0, 64):
                        h = 2 * s + (1 if rowlo else 0)
                        for nch in range(NCH):
                            nc.tensor.matmul(
                                avs[(rowlo, nch)][:],
                                lhsT=vviews[mi][:, h, :],
                                rhs=ets[rowlo][mi][
                                    :, nch * 512 : (nch + 1) * 512
                                ],
                                start=(mi == 0),
                                stop=(mi == CT - 1),
                            )
                for rowlo in (0, 64):
                    for nch in range(NCH):
                        av = avs[(rowlo, nch)]
                        rcp = small.tile([65, 512], fp32)
                        nc.vector.reciprocal_approx_fast(rcp[:], av[:])
                        scr = dscr.tile([1, 512], fp32)
                        nc.sync.dma_start(scr[:], rcp[64:65, :])
                        rb = bcp.tile([64, 512], fp32)
                        nc.gpsimd.dma_start(
                            rb[:], scr[0, :].partition_broadcast(64)
                        )
                        dst = pjs[s][
                            rowlo : rowlo + 64, nch * 512 : (nch + 1) * 512
                        ]
                        if rowlo == 0:
                            nc.vector.tensor_mul(dst, av[0:64, :], rb[:])
                        else:
                            tmp = tmpp.tile([64, 512], fp32r)
                            nc.vector.tensor_mul(tmp[:], av[0:64, :], rb[:])
                            nc.sync.dma_start(dst, tmp[:])

            def emit_scores_exp_mi(s, ets, mi):
                pss = {}
                for rowlo in (0, 64):
                    et = exp_pool.tile([P, N], bf16, name="et", tag="et")
                    ets[rowlo].append(et)
                    pss[rowlo] = psum.tile([P, N], fp32, name="pss", tag="ps")
                for nch in range(NCH):
                    for rowlo in (0, 64):
                        nc.tensor.matmul(
                            pss[rowlo][:, nch * 512 : (nch + 1) * 512],
                            lhsT=kts[s][
                                rowlo : rowlo + 64, mi * P : (mi + 1) * P
                            ],
                            rhs=qts[s][
                                rowlo : rowlo + 64, nch * 512 : (nch + 1) * 512
                            ],
                            start=True,
                            stop=True,
                        )
                for rowlo in (0, 64):
                    nc.scalar.activation(
                        ets[rowlo][mi][:],
                        pss[rowlo][:],
                        AFT.Exp,
                        scale=SCALE,
                    )

            def emit_av_mi(s, ets, avs, mi):
                for rowlo in (0, 64):
                    h = 2 * s + (1 if rowlo else 0)
                    for nch in range(NCH):
                        nc.tensor.matmul(
                            avs[(rowlo, nch)][:],
                            lhsT=vviews[mi][:, h, :],
                            rhs=ets[rowlo][mi][:, nch * 512 : (nch + 1) * 512],
                            start=(mi == 0),
                            stop=(mi == CT - 1),
                        )

            def emit_div(s, avs):
                for rowlo in (0, 64):
                    for nch in range(NCH):
                        av = avs[(rowlo, nch)]
                        rcp = small.tile([65, 512], fp32)
                        nc.vector.reciprocal_approx_fast(rcp[:], av[:])
                        scr = dscr.tile([1, 512], fp32)
                        nc.sync.dma_start(scr[:], rcp[64:65, :])
                        rb = bcp.tile([64, 512], fp32)
                        nc.gpsimd.dma_start(
                            rb[:], scr[0, :].partition_broadcast(64)
                        )
                        dst = pjs[s][
                            rowlo : rowlo + 64, nch * 512 : (nch + 1) * 512
                        ]
                        if rowlo == 0:
                            nc.vector.tensor_mul(dst, av[0:64, :], rb[:])
                        else:
                            tmp = tmpp.tile([64, 512], fp32r)
                            nc.vector.tensor_mul(tmp[:], av[0:64, :], rb[:])
                            nc.sync.dma_start(dst, tmp[:])

            def new_avs():
                return {
                    (rowlo, nch): psav.tile([65, 512], fp32, name="av", tag="av")
                    for rowlo in (0, 64)
                    for nch in range(NCH)
                }

            def emit_pair(s, prev):
                # scores+exp of pair s interleaved (per m-tile) with the AV
                # accumulation of pair prev[0]
                ets = {0: [], 64: []}
                avs = new_avs() if prev is not None else None
                for mi in range(CT):
                    emit_scores_exp_mi(s, ets, mi)
                    if prev is not None:
                        emit_av_mi(prev[0], prev[1], avs, mi)
                if prev is not None:
                    emit_div(prev[0], avs)
                return ets

            tap = os.environ.get("KERNEL_TAP", "")
            run_heads = tap in ("", "pj")
            run_proj = tap == ""
            if run_heads:
                # schedule: x(nch0)+w0 interleaved -> slabs 0,8 -> pair-0
                # scores (ACT starts early) -> rest of qk03 -> v -> pairs
                # 1-2 (+AV 0-1) -> late q/k -> pairs 3-7 -> AV 7 -> proj
                load_x(0)
                wts0 = load_w(0)
                load_x(1)
                wts2 = load_w(2)
                emit_qk_slab(0, wts0, 0)
                emit_qk_slab(8, wts2, 0)
                ets0 = emit_scores_exp(0)
                for ss in range(1, 4):
                    emit_qk_slab(ss, wts0, ss)
                    emit_qk_slab(8 + ss, wts2, ss)
                nc.gpsimd.dma_start(bb[:], bproj.ap().partition_broadcast(P))
                emit_v()
                ets1 = emit_pair(1, (0, ets0))
                ets2 = emit_pair(2, (1, ets1))
                wts1 = load_w(1)
                wts3 = load_w(3)
                for ss in range(4):
                    emit_qk_slab(4 + ss, wts1, ss)
                    emit_qk_slab(12 + ss, wts3, ss)
                prev = (2, ets2)
                for s in range(3, CT):
                    ets = emit_pair(s, prev)
                    prev = (s, ets)
                avs = new_avs()
                for mi in range(CT):
                    emit_av_mi(prev[0], prev[1], avs, mi)
                emit_div(prev[0], avs)
            else:
                load_x(0)
                load_x(1)
                nc.gpsimd.dma_start(bb[:], bproj.ap().partition_broadcast(P))
                wts0 = load_w(0)
                wts2 = load_w(2)
                for ss in range(4):
                    emit_qk_slab(ss, wts0, ss)
                    emit_qk_slab(8 + ss, wts2, ss)
                wts1 = load_w(1)
                wts3 = load_w(3)
                for ss in range(4):
                    emit_qk_slab(4 + ss, wts1, ss)
                    emit_qk_slab(12 + ss, wts3, ss)
                emit_v()

            # ---- phase 4: projection + bias
            if run_proj:
                pwts = []
                for och in range(NCH):
                    wts = []
                    for ci in range(CT):
                        wt = wq.tile([P, 512], fp32r, name="wt", tag="wt")
                        nc.sync.dma_start(
                            wt[:],
                            wprojT.ap()[
                                ci * P : (ci + 1) * P,
                                och * 512 : (och + 1) * 512,
                            ],
                        )
                        wts.append(wt)
                    pwts.append(wts)
                for mi in range(CT):
                    ps = psum.tile([P, N], fp32)
                    for och in range(NCH):
                        for ci in range(CT):
                            nc.tensor.matmul(
                                ps[:, och * 512 : (och + 1) * 512],
                                lhsT=pjs[ci][:, mi * P : (mi + 1) * P],
                                rhs=pwts[och][ci][:],
                                start=(ci == 0),
                                stop=(ci == CT - 1),
                            )
                    ot = otp.tile([P, N], fp32)
                    nc.vector.tensor_add(ot[:], ps[:], bb[:])
                    nc.sync.dma_start(
                        y.ap()[mi * P : (mi + 1) * P, :], ot[:]
                    )

            # ---- debug taps
            if tap in ("q", "k"):
                slabs = qts if tap == "q" else kts
                for s in range(8):
                    ct = otp.tile([P, N], fp32, name="dbgt", tag="dbgt")
                    nc.vector.tensor_copy(ct[:], slabs[s][:])
                    nc.sync.dma_start(y.ap()[s * P : (s + 1) * P, :], ct[:])
            elif tap == "v":
                for mi in range(CT):
                    ct = otp.tile([P, N], fp32, name="dbgt", tag="dbgt")
                    nc.vector.tensor_copy(
                        ct[:].rearrange("p (h d) -> p h d", d=64),
                        vviews[mi][:, :, 0:64],
                    )
                    nc.sync.dma_start(y.ap()[mi * P : (mi + 1) * P, :], ct[:])
            elif tap.startswith("exp"):
                hh = int(tap[3:])
                ets = emit_scores_exp(hh // 2)
                for mi in range(CT):
                    ct = otp.tile([P, N], fp32, name="dbgt", tag="dbgt")
                    nc.vector.tensor_copy(ct[:], ets[(hh % 2) * 64][mi][:])
                    nc.sync.dma_start(y.ap()[mi * P : (mi + 1) * P, :], ct[:])
            elif tap == "pj":
                for s in range(CT):
                    ct = otp.tile([P, N], fp32, name="dbgt", tag="dbgt")
                    nc.vector.tensor_copy(ct[:], pjs[s][:])
                    nc.sync.dma_start(y.ap()[s * P : (s + 1) * P, :], ct[:])

    nc.compile()
    return nc


def kernel(x, w_qkv, w_proj, b_proj):
    global LAST_EXEC_NS
    from concourse.bass_utils import run_bass_kernel_spmd

    x = np.asarray(x, dtype=np.float32)
    w_qkv = np.asarray(w_qkv, dtype=np.float32)
    w_proj = np.asarray(w_proj, dtype=np.float32)
    b_proj = np.asarray(b_proj, dtype=np.float32)

    if "nc" not in _CACHE:
        _CACHE["nc"] = _build()
    nc = _CACHE["nc"]

    wqkvT = np.ascontiguousarray(w_qkv.T)
    wprojT = np.ascontiguousarray(w_proj.T)
    in_maps = [
        {
            "xT": np.ascontiguousarray(x[b].T),
            "wqkvT": wqkvT,
            "wprojT": wprojT,
            "bproj": b_proj,
        }
        for b in range(B)
    ]
    res = run_bass_kernel_spmd(nc, in_maps, core_ids=list(range(B)))
    if res.exec_time_ns is not None:
        LAST_EXEC_NS = res.exec_time_ns
    return np.stack([res.results[b]["y"] for b in range(B)], axis=0)
